# revision 12
# baseline (speedup 1.0000x reference)
"""Trainium2 Bass kernel for a 4-layer GCN (N=50000, D=128, E=1600000, 8 cores).

Strategy (graph/data parallel over destination nodes):
  - Nodes padded to 50176 = 392*128; each of 8 cores owns 6272 nodes (49 tiles).
  - Per layer, linearity lets us reorder:  out = dis * ((sum_{e->v} x'[src_e]) @ W) + b + prev
    with x' = dis * x  (self-loops appended as ordinary edges).
  - x' tables are fp8e4m3, stored as pair-rows: [NPR=25088, 256] where row r
    holds nodes 2r (cols 0:128) and 2r+1 (cols 128:256).  The dma_gather then
    uses 256B-strided rows with 128B payloads (elem_size=128 fp8), with edges
    parity-sorted per destination tile so each gather call uses one column
    half.  128B descriptors halve gather time vs 256B bf16 rows.
  - The scatter-sum runs as: dma_gather of x' rows (fp8, edge-major) + one-hot
    fp8 scatter matrices S generated ON-CHIP (DVE iota==dloc compare),
    accumulated on the PE:  psum_aggT[fi,dst] += msg_chunk[e,fi]^T @ S_chunk[e,dst]
  - aggT (SBUF, bf16) is then lhsT of a second matmul with W moving ->
    node-major psum_out[dst,fo]; epilogue (dis scale, +b+prev, relu) on DVE/ACT.
  - x'_next slices AllGather into the next layer's table in 3 chunks (16/16/17
    tiles) so chunks overlap with the same layer's remaining compute.  The
    table rows are chunk-major-permuted (host-side index remap) so each
    chunked AllGather output is contiguous.
All preprocessing (degree, norm, edge partitioning, permutation, padding) is
host-side numpy; the edge structure is baked into the instruction stream.
"""

import sys

sys.path.insert(0, "/opt/trn_rl_repo")

import numpy as np
import ml_dtypes

N = 50000
D = 128
L = 4
E = 1600000
NCORES = 8
NPAD = 50176  # 392 * 128
NPC = NPAD // NCORES  # 6272 nodes per core
TPC = NPC // 128  # 49 tiles per core
NPR = NPAD // 2  # 25088 pair rows (256B each) in the fp8 table
GROUP = 4  # dst tiles per gather call pair
NGROUPS = (TPC + GROUP - 1) // GROUP  # 13
# collective chunks (in tiles), aligned to GROUP boundaries: groups 0-3 / 4-7 / 8-12
CHUNK_T0 = [0, 16, 32]
CHUNK_T1 = [16, 32, TPC]
CHUNK_LAST_GROUP = [3, 7, NGROUPS - 1]
RPC = [(t1 - t0) * 64 for t0, t1 in zip(CHUNK_T0, CHUNK_T1)]  # pair rows/core/chunk
CHUNK_BASE = [0]
for _r in RPC[:-1]:
    CHUNK_BASE.append(CHUNK_BASE[-1] + NCORES * _r)
# groups whose one-hot S streams from DRAM; the rest generate on DVE.
# Balances DVE busy (~17us/group-gen) against DMA busy (~6us/group-load).
DMA_S_GROUPS = (3, 7, 11)

_compiled = None


def _prow_of(v):
    """Chunk-major permuted pair-row index of node v (vectorized)."""
    c = v // NPC
    r = v % NPC
    t = r // 128
    q = r % 128
    k = np.where(t < CHUNK_T0[1], 0, np.where(t < CHUNK_T0[2], 1, 2))
    base = np.take(np.asarray(CHUNK_BASE, np.int64), k)
    rpc = np.take(np.asarray(RPC, np.int64), k)
    tk0 = np.take(np.asarray(CHUNK_T0, np.int64), k)
    return base + c * rpc + (t - tk0) * 64 + q // 2


def _preprocess(x, edge_index, W, b):
    src0 = edge_index[0].astype(np.int64)
    dst0 = edge_index[1].astype(np.int64)
    loops = np.arange(N, dtype=np.int64)
    src = np.concatenate([src0, loops])
    dst = np.concatenate([dst0, loops])
    deg = np.bincount(dst, minlength=N).astype(np.float32)
    dis = np.zeros(NPAD, np.float32)
    dis[:N] = 1.0 / np.sqrt(deg)

    # order edges by (dst core, dst tile, src parity) once
    core_of = dst // NPC
    lt_of = (dst % NPC) // 128
    dloc_of = dst % 128
    par_of = (src & 1).astype(np.int64)
    key = (core_of * TPC + lt_of) * 2 + par_of
    order = np.argsort(key, kind="stable")
    src_s, key_s = src[order], key[order]
    dloc_s = dloc_of[order]
    prow_s = _prow_of(src_s)
    # counts per (core, tile, parity)
    counts = np.bincount(key_s, minlength=NCORES * TPC * 2).reshape(NCORES, TPC, 2)
    # shared chunk structure: per (tile, parity) max over cores
    KCH = np.ceil(counts.max(axis=0) / 128.0).astype(np.int64)  # [TPC, 2]
    KCH = np.maximum(KCH, 1)

    # group layout (shared across cores): group g covers tiles [g*GROUP, ...)
    # chunk stream order per group: even chunks of its tiles, then odd chunks
    tiles_of_group = [list(range(g * GROUP, min((g + 1) * GROUP, TPC))) for g in range(NGROUPS)]
    chE = [int(sum(KCH[t, 0] for t in tg)) for tg in tiles_of_group]
    chO = [int(sum(KCH[t, 1] for t in tg)) for tg in tiles_of_group]
    TOTCH = int(sum(chE) + sum(chO))

    chunk_off = np.zeros((TPC, 2), np.int64)
    pos = 0
    for g, tg in enumerate(tiles_of_group):
        for h in (0, 1):
            for t in tg:
                chunk_off[t, h] = pos
                pos += KCH[t, h]
    assert pos == TOTCH

    # per-core idx (pair rows) + dloc streams
    idx_all = np.zeros((NCORES, TOTCH * 128), np.int16)
    dlocs = np.full((NCORES, TOTCH * 128), -1, np.int16)
    starts = np.zeros(NCORES * TPC * 2 + 1, np.int64)
    np.cumsum(counts.reshape(-1), out=starts[1:])
    for c in range(NCORES):
        for t in range(TPC):
            for h in (0, 1):
                k = (c * TPC + t) * 2 + h
                s0, s1 = starts[k], starts[k + 1]
                n = s1 - s0
                o = chunk_off[t, h] * 128
                idx_all[c, o:o + n] = prow_s[s0:s1].astype(np.int16)
                dlocs[c, o:o + n] = dloc_s[s0:s1]

    # idx tiles wrapped: element i at [i%16, i//16], replicated x8 partition groups
    idxw = np.zeros((NCORES, 128, TOTCH * 8), np.int16)
    for c in range(NCORES):
        w = idx_all[c].reshape(TOTCH * 8, 16).T
        for gme in range(8):
            idxw[c, gme * 16:(gme + 1) * 16, :] = w
    # dloc wrapped for on-chip S generation: [128, TOTCH], slot p of chunk ch
    dlocw = dlocs.reshape(NCORES, TOTCH, 128).transpose(0, 2, 1).copy()

    # pre-built one-hot S (transposed layout) for the DMA-sourced groups
    dma_cols = []
    smat_off = {}
    pos_s = 0
    for g in DMA_S_GROUPS:
        tg = tiles_of_group[g]
        goff = int(chunk_off[tg[0], 0])
        nEO = chE[g] + chO[g]
        dma_cols.extend(range(goff, goff + nEO))
        smat_off[g] = pos_s
        pos_s += nEO
    CHD = pos_s
    dma_cols = np.asarray(dma_cols, np.int64)
    jj = np.arange(128, dtype=np.int16)
    smat_t = np.empty((NCORES, 128, CHD, 128), ml_dtypes.float8_e4m3)
    for c in range(NCORES):
        smat_t[c] = (dlocw[c][:, dma_cols, None] == jj[None, None, :])

    # fp8 x' table in permuted pair-row layout
    xpad = np.zeros((NPAD, D), np.float32)
    xpad[:N] = x
    xp8 = (xpad * dis[:, None]).astype(ml_dtypes.float8_e4m3)
    allv = np.arange(NPAD, dtype=np.int64)
    pos8 = _prow_of(allv) * 2 + (allv & 1)
    x0p = np.zeros((NPR * 2, D), ml_dtypes.float8_e4m3)
    x0p[pos8] = xp8
    x0p = x0p.reshape(NPR, 256)

    disT = dis.reshape(NCORES, TPC, 128).transpose(0, 2, 1).copy()  # [c,128,TPC]
    b_bc = np.broadcast_to(b[None, :, :], (128, L, D)).astype(np.float32).copy()

    meta = dict(KCH=KCH, chE=chE, chO=chO, TOTCH=TOTCH, chunk_off=chunk_off,
                tiles_of_group=tiles_of_group, smat_off=smat_off, CHD=CHD)
    per_core = []
    for c in range(NCORES):
        per_core.append(dict(
            x_own=np.ascontiguousarray(xpad[c * NPC:(c + 1) * NPC]),
            x0p=x0p,
            w=W.astype(ml_dtypes.bfloat16),
            b_bc=b_bc,
            disT=np.ascontiguousarray(disT[c]),
            idxs=np.ascontiguousarray(idxw[c]),
            dloc=np.ascontiguousarray(dlocw[c]),
            smat=np.ascontiguousarray(smat_t[c]),
        ))
    return meta, per_core


def _dma_gather_raw(eng, mybir, out_ap, in_ap, idxs_ap, num_idxs, elem_size,
                    single_packet=False):
    """bass.dma_gather without the elem_size%256 restriction (stride must
    still be a 256B multiple).  in_ap rows: [nrows, elem_step] with the
    gathered payload in the leading elem_size elements."""
    stride_bytes = in_ap.ap[0][0] * mybir.dt.size(in_ap.dtype)
    stride_bytes_256 = stride_bytes // 256
    assert stride_bytes % 256 == 0 and 0 < stride_bytes_256 < 256
    _in_ap = eng.lower_ap_dma(in_ap, for_custom_bir_dma=True)
    _idxs_ap = eng.lower_ap(idxs_ap)
    _out_ap = eng.lower_ap(out_ap)
    return eng.add_instruction(
        mybir.InstDMAGatherAnt(
            name=eng.bass.get_next_instruction_name(),
            ins=[*_in_ap, _idxs_ap, eng.lower_val_access(eng.to_reg(num_idxs))],
            outs=[_out_ap],
            transpose=False,
            num_idxs=num_idxs,
            elem_size=elem_size,
            stride_bytes_256=stride_bytes_256,
            gen_mode=0,
            single_packet=single_packet,
            queue_num=0,
            sbuf_tokens_per_rank=0,
            sbuf_free_dim_per_rank=0,
            sbuf_free_dim_pad_per_rank=0,
            sbuf_byte_offset=0,
        ))


def _build(meta):
    from concourse import bacc, tile, bass_utils
    from concourse.bass import mybir

    KCH = meta["KCH"]
    chE, chO = meta["chE"], meta["chO"]
    TOTCH = meta["TOTCH"]
    chunk_off = meta["chunk_off"]
    tiles_of_group = meta["tiles_of_group"]
    smat_off = meta["smat_off"]
    CHD = meta["CHD"]
    MAXCH_G = max(chE[g] + chO[g] for g in range(NGROUPS))

    nc = bacc.Bacc("TRN2", target_bir_lowering=False, debug=False,
                   num_devices=NCORES)
    d_x_own = nc.dram_tensor("x_own", [NPC, D], mybir.dt.float32, kind="ExternalInput")
    d_x0p = nc.dram_tensor("x0p", [NPR, 256], mybir.dt.float8e4, kind="ExternalInput")
    d_w = nc.dram_tensor("w", [L, D, D], mybir.dt.bfloat16, kind="ExternalInput")
    d_bbc = nc.dram_tensor("b_bc", [128, L, D], mybir.dt.float32, kind="ExternalInput")
    d_disT = nc.dram_tensor("disT", [128, TPC], mybir.dt.float32, kind="ExternalInput")
    d_idxs = nc.dram_tensor("idxs", [128, TOTCH * 8], mybir.dt.int16, kind="ExternalInput")
    d_dloc = nc.dram_tensor("dloc", [128, TOTCH], mybir.dt.int16, kind="ExternalInput")
    d_smat = nc.dram_tensor("smat", [128, CHD, 128], mybir.dt.float8e4, kind="ExternalInput")
    d_out = nc.dram_tensor("x_out", [NPC, D], mybir.dt.float32, kind="ExternalOutput")

    with tile.TileContext(nc) as tc:
        with (
            tc.tile_pool(name="const", bufs=1) as constp,
            tc.tile_pool(name="msg", bufs=3) as msgp,
            tc.tile_pool(name="sp", bufs=3) as sp_pool,
            tc.tile_pool(name="work", bufs=3) as workp,
            tc.tile_pool(name="xprime", bufs=TPC + 1) as xpp,
            tc.tile_pool(name="pag", bufs=2, space="PSUM") as pagp,
            tc.tile_pool(name="pout", bufs=2, space="PSUM") as poutp,
            tc.tile_pool(name="dram", bufs=1, space="DRAM") as dramp,
        ):
            # ---- persistent SBUF state ----
            x_own = constp.tile([128, TPC, D], mybir.dt.float32, tag="x_own")
            nc.sync.dma_start(out=x_own[:], in_=d_x_own.ap().rearrange("(t p) f -> p t f", p=128))
            w_sb = constp.tile([128, L, D], mybir.dt.bfloat16, tag="w_sb")
            nc.sync.dma_start(out=w_sb[:], in_=d_w.ap().rearrange("l k f -> k l f"))
            bbc_sb = constp.tile([128, L, D], mybir.dt.float32, tag="bbc")
            nc.sync.dma_start(out=bbc_sb[:], in_=d_bbc.ap())
            disT_sb = constp.tile([128, TPC], mybir.dt.float32, tag="disT")
            nc.sync.dma_start(out=disT_sb[:], in_=d_disT.ap())
            idx_sb = constp.tile([128, TOTCH * 8], mybir.dt.int16, tag="idx")
            nc.sync.dma_start(out=idx_sb[:], in_=d_idxs.ap())
            dloc_sb = constp.tile([128, TOTCH], mybir.dt.int16, tag="dloc")
            nc.sync.dma_start(out=dloc_sb[:], in_=d_dloc.ap())
            iota_sb = constp.tile([128, 128], mybir.dt.int16, tag="iota")
            nc.gpsimd.iota(iota_sb[:], pattern=[[1, 128]], base=0,
                           channel_multiplier=0)

            # DRAM tables for layers 1..3 and AllGather input slices
            tables = [d_x0p.ap()]
            ag_ins = []
            for l in range(1, L):
                tab_tile = dramp.tile([NPR, 256], mybir.dt.float8e4, tag=f"tab{l}")
                agin_tile = dramp.tile([NPC, D], mybir.dt.float8e4, tag=f"agin{l}")
                tables.append(tab_tile[:])
                ag_ins.append(agin_tile[:])

            for l in range(L):
                table = tables[l]
                for g in range(NGROUPS):
                    tg = tiles_of_group[g]
                    nE, nO = chE[g], chO[g]
                    goff = int(chunk_off[tg[0], 0])  # stream offset of this group
                    # one-hot S for the whole group: DMA-streamed or DVE-generated
                    s_t = sp_pool.tile([128, MAXCH_G, 128], mybir.dt.float8e4, tag="s_t")
                    if g in DMA_S_GROUPS:
                        nc.sync.dma_start(
                            out=s_t[:, 0:nE + nO, :],
                            in_=d_smat.ap()[:, smat_off[g]:smat_off[g] + nE + nO, :])
                    else:
                        nc.vector.tensor_tensor(
                            out=s_t[:, 0:nE + nO, :],
                            in0=dloc_sb[:, goff:goff + nE + nO].broadcast_to(
                                [128, nE + nO, 128]),
                            in1=iota_sb[:].rearrange("p j -> p () j").broadcast_to(
                                [128, nE + nO, 128]),
                            op=mybir.AluOpType.is_equal)
                    # gather msg rows for even/odd source parity
                    msg = msgp.tile([128, MAXCH_G, 128], mybir.dt.float8e4, tag="msg")
                    if nE > 0:
                        _dma_gather_raw(
                            nc.gpsimd, mybir,
                            out_ap=msg[:, 0:nE, :],
                            in_ap=table[:, 0:128],
                            idxs_ap=idx_sb[:, goff * 8:(goff + nE) * 8],
                            num_idxs=nE * 128, elem_size=128)
                    if nO > 0:
                        _dma_gather_raw(
                            nc.gpsimd, mybir,
                            out_ap=msg[:, nE:nE + nO, :],
                            in_ap=table[:, 128:256],
                            idxs_ap=idx_sb[:, (goff + nE) * 8:(goff + nE + nO) * 8],
                            num_idxs=nO * 128, elem_size=128)
                    # chunk k's AllGather goes on the Pool queue one group
                    # after its last tile, so it doesn't stall the next
                    # group's gather prefetch while waiting on x' writes
                    if l < L - 1:
                        for k in range(len(CHUNK_T0)):
                            if g == CHUNK_LAST_GROUP[k] + 1:
                                nc.gpsimd.collective_compute(
                                    "AllGather",
                                    mybir.AluOpType.bypass,
                                    replica_groups=[list(range(NCORES))],
                                    ins=[ag_ins[l][CHUNK_T0[k] * 128:CHUNK_T1[k] * 128, :].opt()],
                                    outs=[tables[l + 1][CHUNK_BASE[k]:CHUNK_BASE[k] + NCORES * RPC[k], :].opt()],
                                )
                    # per dst tile: accumulate one-hot matmuls, then @W + epilogue
                    for t in tg:
                        kE = int(KCH[t, 0])
                        kO = int(KCH[t, 1])
                        oE = int(chunk_off[t, 0] - goff)
                        oO = int(chunk_off[t, 1] - goff)
                        psA = pagp.tile([128, 128], mybir.dt.float32, tag="pag")
                        nchunks = kE + kO
                        ci = 0
                        for c in range(kE):
                            nc.tensor.matmul(psA[:], lhsT=msg[:, oE + c, :],
                                             rhs=s_t[:, oE + c, :],
                                             start=(ci == 0), stop=(ci == nchunks - 1))
                            ci += 1
                        for c in range(kO):
                            nc.tensor.matmul(psA[:], lhsT=msg[:, oO + c, :],
                                             rhs=s_t[:, oO + c, :],
                                             start=(ci == 0), stop=(ci == nchunks - 1))
                            ci += 1
                        aggT = workp.tile([128, 128], mybir.dt.bfloat16, tag="aggT")
                        nc.scalar.copy(aggT[:], psA[:])
                        pso = poutp.tile([128, 128], mybir.dt.float32, tag="pout")
                        nc.tensor.matmul(pso[:], lhsT=aggT[:], rhs=w_sb[:, l, :],
                                         start=True, stop=True)
                        # epilogue: xn = relu(dis*pso + b + prev); x' = fp8(dis*xn)
                        prevb = workp.tile([128, 128], mybir.dt.float32, tag="prevb")
                        nc.vector.tensor_tensor(
                            out=prevb[:], in0=x_own[:, t, :], in1=bbc_sb[:, l, :],
                            op=mybir.AluOpType.add)
                        t2 = workp.tile([128, 128], mybir.dt.float32, tag="t2")
                        nc.vector.tensor_scalar(
                            out=t2[:], in0=pso[:], scalar1=disT_sb[:, t:t + 1],
                            scalar2=None, op0=mybir.AluOpType.mult)
                        t3 = workp.tile([128, 128], mybir.dt.float32, tag="t3")
                        nc.vector.tensor_tensor(
                            out=t3[:], in0=t2[:], in1=prevb[:], op=mybir.AluOpType.add)
                        nc.scalar.activation(
                            out=x_own[:, t, :], in_=t3[:],
                            func=mybir.ActivationFunctionType.Relu)
                        if l < L - 1:
                            xpr = xpp.tile([128, 128], mybir.dt.float8e4, tag="xpr")
                            nc.scalar.activation(
                                out=xpr[:], in_=x_own[:, t, :],
                                func=mybir.ActivationFunctionType.Copy,
                                scale=disT_sb[:, t:t + 1])
                            nc.sync.dma_start(
                                out=ag_ins[l].rearrange("(t p) f -> p t f", p=128)[:, t, :],
                                in_=xpr[:])
                # the last chunk's AllGather fires at end of layer
                if l < L - 1:
                    k = len(CHUNK_T0) - 1
                    nc.gpsimd.collective_compute(
                        "AllGather",
                        mybir.AluOpType.bypass,
                        replica_groups=[list(range(NCORES))],
                        ins=[ag_ins[l][CHUNK_T0[k] * 128:CHUNK_T1[k] * 128, :].opt()],
                        outs=[tables[l + 1][CHUNK_BASE[k]:CHUNK_BASE[k] + NCORES * RPC[k], :].opt()],
                    )
            nc.sync.dma_start(out=d_out.ap().rearrange("(t p) f -> p t f", p=128),
                              in_=x_own[:])

    nc.compile()
    return nc


def kernel(x, edge_index, W, b):
    global _compiled
    from concourse import bass_utils

    x = np.asarray(x, dtype=np.float32)
    W_np = np.asarray(W, dtype=np.float32)
    b_np = np.asarray(b, dtype=np.float32)
    ei = np.asarray(edge_index)

    meta, per_core = _preprocess(x, ei, W_np, b_np)
    globals()["_last_per_core"] = per_core
    if _compiled is None:
        _compiled = _build(meta)
    nc = _compiled
    res = bass_utils.run_bass_kernel_spmd(nc, per_core, core_ids=list(range(NCORES)))
    out = np.concatenate([res.results[c]["x_out"] for c in range(NCORES)], axis=0)
    return out[:N].astype(np.float32)


# revision 32
# speedup vs baseline: 1.0786x; 1.0786x over previous
"""Trainium2 Bass kernel for a 4-layer GCN (N=50000, D=128, E=1600000, 8 cores).

Strategy (graph/data parallel over destination nodes):
  - Nodes padded to 50176 = 392*128; each of 8 cores owns 6272 nodes (49 tiles).
  - Per layer, linearity lets us reorder:  out = dis * ((sum_{e->v} x'[src_e]) @ W) + b + prev
    with x' = dis * x  (self-loops appended as ordinary edges).
  - x' tables are fp8e4m3, stored as pair-rows: [NPR=25088, 256] where row r
    holds nodes 2r (cols 0:128) and 2r+1 (cols 128:256).  The dma_gather then
    uses 256B-strided rows with 128B payloads (elem_size=128 fp8), with edges
    parity-sorted per destination tile so each gather call uses one column
    half.  128B descriptors halve gather time vs 256B bf16 rows.
  - The scatter-sum runs as: dma_gather of x' rows (fp8, edge-major) + one-hot
    fp8 scatter matrices S generated ON-CHIP (DVE iota==dloc compare),
    accumulated on the PE:  psum_aggT[fi,dst] += msg_chunk[e,fi]^T @ S_chunk[e,dst]
  - aggT (SBUF, bf16) is then lhsT of a second matmul with W moving ->
    node-major psum_out[dst,fo]; epilogue (dis scale, +b+prev, relu) on DVE/ACT.
  - x'_next slices AllGather into the next layer's table in 3 chunks (16/16/17
    tiles) so chunks overlap with the same layer's remaining compute.  The
    table rows are chunk-major-permuted (host-side index remap) so each
    chunked AllGather output is contiguous.
All preprocessing (degree, norm, edge partitioning, permutation, padding) is
host-side numpy; the edge structure is baked into the instruction stream.
"""

import sys

sys.path.insert(0, "/opt/trn_rl_repo")

import numpy as np
import ml_dtypes

N = 50000
D = 128
L = 4
E = 1600000
NCORES = 8
NPAD = 50176  # 392 * 128
NPC = NPAD // NCORES  # 6272 nodes per core
TPC = NPC // 128  # 49 tiles per core
NPR = NPAD // 2  # 25088 pair rows (256B each) in the fp8 table
GROUP = 4  # dst tiles per gather call pair
NGROUPS = (TPC + GROUP - 1) // GROUP  # 13
# collective chunks (in tiles), aligned to GROUP boundaries: groups 0-3 / 4-7 / 8-12
CHUNK_T0 = [0, 16, 32]
CHUNK_T1 = [16, 32, TPC]
CHUNK_LAST_GROUP = [3, 7, NGROUPS - 1]
RPC = [(t1 - t0) * 64 for t0, t1 in zip(CHUNK_T0, CHUNK_T1)]  # pair rows/core/chunk
CHUNK_BASE = [0]
for _r in RPC[:-1]:
    CHUNK_BASE.append(CHUNK_BASE[-1] + NCORES * _r)
# groups whose one-hot S streams from DRAM; the rest generate on DVE.
# Balances DVE busy (~17us/group-gen) against DMA busy (~6us/group-load).
DMA_S_GROUPS = (2, 5, 8, 11)

_compiled = None


def _prow_of(v):
    """Chunk-major permuted pair-row index of node v (vectorized)."""
    c = v // NPC
    r = v % NPC
    t = r // 128
    q = r % 128
    k = np.searchsorted(np.asarray(CHUNK_T0, np.int64), t, side="right") - 1
    base = np.take(np.asarray(CHUNK_BASE, np.int64), k)
    rpc = np.take(np.asarray(RPC, np.int64), k)
    tk0 = np.take(np.asarray(CHUNK_T0, np.int64), k)
    return base + c * rpc + (t - tk0) * 64 + q // 2


def _preprocess(x, edge_index, W, b):
    # self-loops are NOT added to the edge stream: they are applied on-chip
    # as an identity matmul over the SBUF-resident x' tiles.  deg still
    # counts them (A_hat = A + I normalization).
    src = edge_index[0].astype(np.int64)
    dst = edge_index[1].astype(np.int64)
    deg = np.bincount(dst, minlength=N).astype(np.float32) + 1.0
    dis = np.zeros(NPAD, np.float32)
    dis[:N] = 1.0 / np.sqrt(deg)

    # order edges by (dst core, dst tile, src parity) once
    core_of = dst // NPC
    lt_of = (dst % NPC) // 128
    dloc_of = dst % 128
    par_of = (src & 1).astype(np.int64)
    key = (core_of * TPC + lt_of) * 2 + par_of
    order = np.argsort(key, kind="stable")
    src_s, key_s = src[order], key[order]
    dloc_s = dloc_of[order]
    prow_s = _prow_of(src_s)
    # counts per (core, tile, parity)
    counts = np.bincount(key_s, minlength=NCORES * TPC * 2).reshape(NCORES, TPC, 2)
    # shared chunk structure: per (tile, parity) max over cores
    KCH = np.ceil(counts.max(axis=0) / 128.0).astype(np.int64)  # [TPC, 2]
    KCH = np.maximum(KCH, 1)

    # group layout (shared across cores): group g covers tiles [g*GROUP, ...)
    # chunk stream order per group: even chunks of its tiles, then odd chunks
    tiles_of_group = [list(range(g * GROUP, min((g + 1) * GROUP, TPC))) for g in range(NGROUPS)]
    chE = [int(sum(KCH[t, 0] for t in tg)) for tg in tiles_of_group]
    chO = [int(sum(KCH[t, 1] for t in tg)) for tg in tiles_of_group]
    TOTCH = int(sum(chE) + sum(chO))

    chunk_off = np.zeros((TPC, 2), np.int64)
    pos = 0
    for g, tg in enumerate(tiles_of_group):
        for h in (0, 1):
            for t in tg:
                chunk_off[t, h] = pos
                pos += KCH[t, h]
    assert pos == TOTCH

    # per-core idx (pair rows) + dloc streams
    idx_all = np.zeros((NCORES, TOTCH * 128), np.int16)
    dlocs = np.full((NCORES, TOTCH * 128), -1, np.int16)
    starts = np.zeros(NCORES * TPC * 2 + 1, np.int64)
    np.cumsum(counts.reshape(-1), out=starts[1:])
    for c in range(NCORES):
        for t in range(TPC):
            for h in (0, 1):
                k = (c * TPC + t) * 2 + h
                s0, s1 = starts[k], starts[k + 1]
                n = s1 - s0
                o = chunk_off[t, h] * 128
                idx_all[c, o:o + n] = prow_s[s0:s1].astype(np.int16)
                dlocs[c, o:o + n] = dloc_s[s0:s1]

    # idx tiles wrapped: element i at [i%16, i//16], replicated x8 partition groups
    idxw = np.zeros((NCORES, 128, TOTCH * 8), np.int16)
    for c in range(NCORES):
        w = idx_all[c].reshape(TOTCH * 8, 16).T
        for gme in range(8):
            idxw[c, gme * 16:(gme + 1) * 16, :] = w
    # dloc wrapped for on-chip S generation: [128, TOTCH], slot p of chunk ch
    dlocw = dlocs.reshape(NCORES, TOTCH, 128).transpose(0, 2, 1).copy()

    # pre-built one-hot S (transposed layout) for the DMA-sourced groups
    dma_cols = []
    smat_off = {}
    pos_s = 0
    for g in DMA_S_GROUPS:
        tg = tiles_of_group[g]
        goff = int(chunk_off[tg[0], 0])
        nEO = chE[g] + chO[g]
        dma_cols.extend(range(goff, goff + nEO))
        smat_off[g] = pos_s
        pos_s += nEO
    CHD = pos_s
    dma_cols = np.asarray(dma_cols, np.int64)
    jj = np.arange(128, dtype=np.int16)
    smat_t = np.empty((NCORES, 128, CHD, 128), ml_dtypes.float8_e4m3)
    for c in range(NCORES):
        smat_t[c] = (dlocw[c][:, dma_cols, None] == jj[None, None, :])

    # fp8 x' table in permuted pair-row layout
    xpad = np.zeros((NPAD, D), np.float32)
    xpad[:N] = x
    xp8 = (xpad * dis[:, None]).astype(ml_dtypes.float8_e4m3)
    allv = np.arange(NPAD, dtype=np.int64)
    pos8 = _prow_of(allv) * 2 + (allv & 1)
    x0p = np.zeros((NPR * 2, D), ml_dtypes.float8_e4m3)
    x0p[pos8] = xp8
    x0p = x0p.reshape(NPR, 256)

    disT = dis.reshape(NCORES, TPC, 128).transpose(0, 2, 1).copy()  # [c,128,TPC]
    b1 = b[None, :, :].astype(ml_dtypes.bfloat16)  # [1, L, D] for rank-1 psum add

    meta = dict(KCH=KCH, chE=chE, chO=chO, TOTCH=TOTCH, chunk_off=chunk_off,
                tiles_of_group=tiles_of_group, smat_off=smat_off, CHD=CHD)
    per_core = []
    for c in range(NCORES):
        per_core.append(dict(
            x_own=np.ascontiguousarray(xpad[c * NPC:(c + 1) * NPC]),
            x0p=x0p,
            w=W.astype(ml_dtypes.bfloat16),
            b1=b1,
            disT=np.ascontiguousarray(disT[c]),
            idxs=np.ascontiguousarray(idxw[c]),
            dloc=np.ascontiguousarray(dlocw[c]),
            smat=np.ascontiguousarray(smat_t[c]),
        ))
    return meta, per_core


def _dma_gather_raw(eng, mybir, out_ap, in_ap, idxs_ap, num_idxs, elem_size,
                    single_packet=False):
    """bass.dma_gather without the elem_size%256 restriction (stride must
    still be a 256B multiple).  in_ap rows: [nrows, elem_step] with the
    gathered payload in the leading elem_size elements."""
    stride_bytes = in_ap.ap[0][0] * mybir.dt.size(in_ap.dtype)
    stride_bytes_256 = stride_bytes // 256
    assert stride_bytes % 256 == 0 and 0 < stride_bytes_256 < 256
    _in_ap = eng.lower_ap_dma(in_ap, for_custom_bir_dma=True)
    _idxs_ap = eng.lower_ap(idxs_ap)
    _out_ap = eng.lower_ap(out_ap)
    return eng.add_instruction(
        mybir.InstDMAGatherAnt(
            name=eng.bass.get_next_instruction_name(),
            ins=[*_in_ap, _idxs_ap, eng.lower_val_access(eng.to_reg(num_idxs))],
            outs=[_out_ap],
            transpose=False,
            num_idxs=num_idxs,
            elem_size=elem_size,
            stride_bytes_256=stride_bytes_256,
            gen_mode=0,
            single_packet=single_packet,
            queue_num=0,
            sbuf_tokens_per_rank=0,
            sbuf_free_dim_per_rank=0,
            sbuf_free_dim_pad_per_rank=0,
            sbuf_byte_offset=0,
        ))


def _build(meta):
    from concourse import bacc, tile, bass_utils
    from concourse.bass import mybir

    KCH = meta["KCH"]
    chE, chO = meta["chE"], meta["chO"]
    TOTCH = meta["TOTCH"]
    chunk_off = meta["chunk_off"]
    tiles_of_group = meta["tiles_of_group"]
    smat_off = meta["smat_off"]
    CHD = meta["CHD"]
    MAXCH_G = max(chE[g] + chO[g] for g in range(NGROUPS))

    nc = bacc.Bacc("TRN2", target_bir_lowering=False, debug=False,
                   num_devices=NCORES)
    d_x_own = nc.dram_tensor("x_own", [NPC, D], mybir.dt.float32, kind="ExternalInput")
    d_x0p = nc.dram_tensor("x0p", [NPR, 256], mybir.dt.float8e4, kind="ExternalInput")
    d_w = nc.dram_tensor("w", [L, D, D], mybir.dt.bfloat16, kind="ExternalInput")
    d_b1 = nc.dram_tensor("b1", [1, L, D], mybir.dt.bfloat16, kind="ExternalInput")
    d_disT = nc.dram_tensor("disT", [128, TPC], mybir.dt.float32, kind="ExternalInput")
    d_idxs = nc.dram_tensor("idxs", [128, TOTCH * 8], mybir.dt.int16, kind="ExternalInput")
    d_dloc = nc.dram_tensor("dloc", [128, TOTCH], mybir.dt.int16, kind="ExternalInput")
    d_smat = nc.dram_tensor("smat", [128, CHD, 128], mybir.dt.float8e4, kind="ExternalInput")
    d_out = nc.dram_tensor("x_out", [NPC, D], mybir.dt.float32, kind="ExternalOutput")

    with tile.TileContext(nc) as tc:
        with (
            tc.tile_pool(name="const", bufs=1) as constp,
            tc.tile_pool(name="msg", bufs=4) as msgp,
            tc.tile_pool(name="sp", bufs=3) as sp_pool,
            tc.tile_pool(name="work", bufs=3) as workp,

            tc.tile_pool(name="pag", bufs=2, space="PSUM") as pagp,
            tc.tile_pool(name="pout", bufs=2, space="PSUM") as poutp,
            tc.tile_pool(name="dram", bufs=1, space="DRAM") as dramp,
        ):
            # ---- persistent SBUF state ----
            x_own = constp.tile([128, TPC, D], mybir.dt.float32, tag="x_own")
            nc.sync.dma_start(out=x_own[:], in_=d_x_own.ap().rearrange("(t p) f -> p t f", p=128))
            w_sb = constp.tile([128, L, D], mybir.dt.bfloat16, tag="w_sb")
            nc.sync.dma_start(out=w_sb[:], in_=d_w.ap().rearrange("l k f -> k l f"))
            b1_sb = constp.tile([1, L, D], mybir.dt.bfloat16, tag="b1")
            nc.sync.dma_start(out=b1_sb[:], in_=d_b1.ap())
            ones1 = constp.tile([1, 128], mybir.dt.bfloat16, tag="ones1")
            nc.gpsimd.memset(ones1[:], 1.0)
            disT_sb = constp.tile([128, TPC], mybir.dt.float32, tag="disT")
            nc.sync.dma_start(out=disT_sb[:], in_=d_disT.ap())
            idx_sb = constp.tile([128, TOTCH * 8], mybir.dt.int16, tag="idx")
            nc.sync.dma_start(out=idx_sb[:], in_=d_idxs.ap())
            dloc_sb = constp.tile([128, TOTCH], mybir.dt.int16, tag="dloc")
            nc.sync.dma_start(out=dloc_sb[:], in_=d_dloc.ap())
            iota_sb = constp.tile([128, 128], mybir.dt.int16, tag="iota")
            nc.gpsimd.iota(iota_sb[:], pattern=[[1, 128]], base=0,
                           channel_multiplier=0)
            piota_sb = constp.tile([128, 128], mybir.dt.int16, tag="piota")
            nc.gpsimd.iota(piota_sb[:], pattern=[[0, 128]], base=0,
                           channel_multiplier=1)
            ident_sb = constp.tile([128, 128], mybir.dt.float8e4, tag="ident")
            nc.vector.tensor_tensor(out=ident_sb[:], in0=piota_sb[:],
                                    in1=iota_sb[:], op=mybir.AluOpType.is_equal)
            # persistent x' (fp8, dis*x) of the current layer, per own tile
            xp_sb = constp.tile([128, TPC, 128], mybir.dt.float8e4, tag="xp_sb")
            for t in range(TPC):
                nc.scalar.activation(
                    out=xp_sb[:, t, :], in_=x_own[:, t, :],
                    func=mybir.ActivationFunctionType.Copy,
                    scale=disT_sb[:, t:t + 1])

            # DRAM tables for layers 1..3 and AllGather input slices
            tables = [d_x0p.ap()]
            ag_ins = []
            for l in range(1, L):
                tab_tile = dramp.tile([NPR, 256], mybir.dt.float8e4, tag=f"tab{l}")
                agin_tile = dramp.tile([NPC, D], mybir.dt.float8e4, tag=f"agin{l}")
                tables.append(tab_tile[:])
                ag_ins.append(agin_tile[:])

            for l in range(L):
                table = tables[l]
                for g in range(NGROUPS):
                    tg = tiles_of_group[g]
                    nE, nO = chE[g], chO[g]
                    goff = int(chunk_off[tg[0], 0])  # stream offset of this group
                    # one-hot S for the whole group: DMA-streamed or DVE-generated
                    # (generated in per-(tile,parity) slices so consuming
                    # matmuls start before the whole group's S is ready)
                    s_t = sp_pool.tile([128, MAXCH_G, 128], mybir.dt.float8e4, tag="s_t")
                    if g in DMA_S_GROUPS:
                        nc.sync.dma_start(
                            out=s_t[:, 0:nE + nO, :],
                            in_=d_smat.ap()[:, smat_off[g]:smat_off[g] + nE + nO, :])
                    else:
                        for t in tg:
                            for h in (0, 1):
                                o0 = int(chunk_off[t, h] - goff)
                                kk = int(KCH[t, h])
                                nc.vector.tensor_tensor(
                                    out=s_t[:, o0:o0 + kk, :],
                                    in0=dloc_sb[:, goff + o0:goff + o0 + kk].broadcast_to(
                                        [128, kk, 128]),
                                    in1=iota_sb[:].rearrange("p j -> p () j").broadcast_to(
                                        [128, kk, 128]),
                                    op=mybir.AluOpType.is_equal)
                    # gather msg rows for even/odd source parity
                    msg = msgp.tile([128, MAXCH_G, 128], mybir.dt.float8e4, tag="msg")
                    if nE > 0:
                        _dma_gather_raw(
                            nc.gpsimd, mybir,
                            out_ap=msg[:, 0:nE, :],
                            in_ap=table[:, 0:128],
                            idxs_ap=idx_sb[:, goff * 8:(goff + nE) * 8],
                            num_idxs=nE * 128, elem_size=128)
                    if nO > 0:
                        _dma_gather_raw(
                            nc.gpsimd, mybir,
                            out_ap=msg[:, nE:nE + nO, :],
                            in_ap=table[:, 128:256],
                            idxs_ap=idx_sb[:, (goff + nE) * 8:(goff + nE + nO) * 8],
                            num_idxs=nO * 128, elem_size=128)
                    # chunk k's AllGather goes on the Pool queue one group
                    # after its last tile, so it doesn't stall the next
                    # group's gather prefetch while waiting on x' writes
                    if l < L - 1:
                        for k in range(len(CHUNK_T0)):
                            if g == CHUNK_LAST_GROUP[k] + 1:
                                nc.gpsimd.collective_compute(
                                    "AllGather",
                                    mybir.AluOpType.bypass,
                                    replica_groups=[list(range(NCORES))],
                                    ins=[ag_ins[l][CHUNK_T0[k] * 128:CHUNK_T1[k] * 128, :].opt()],
                                    outs=[tables[l + 1][CHUNK_BASE[k]:CHUNK_BASE[k] + NCORES * RPC[k], :].opt()],
                                )
                    # per dst tile: accumulate one-hot matmuls, then @W + epilogue
                    for t in tg:
                        kE = int(KCH[t, 0])
                        kO = int(KCH[t, 1])
                        oE = int(chunk_off[t, 0] - goff)
                        oO = int(chunk_off[t, 1] - goff)
                        psA = pagp.tile([128, 128], mybir.dt.float32, tag="pag")
                        # DoubleRow fp8: one instruction accumulates two
                        # 128-edge chunks (k-tile pairs); odd tail is regular.
                        # The final matmul adds the self-loop term x'[t] @ I.
                        nmm = (kE + 1) // 2 + (kO + 1) // 2 + 1
                        ci = 0
                        for o, k in ((oE, kE), (oO, kO)):
                            for c in range(0, k - 1, 2):
                                nc.tensor.matmul(
                                    psA[:], lhsT=msg[:, o + c:o + c + 2, :],
                                    rhs=s_t[:, o + c:o + c + 2, :],
                                    start=(ci == 0), stop=(ci == nmm - 1),
                                    perf_mode=mybir.MatmulPerfMode.DoubleRow)
                                ci += 1
                            if k % 2:
                                nc.tensor.matmul(
                                    psA[:], lhsT=msg[:, o + k - 1, :],
                                    rhs=s_t[:, o + k - 1, :],
                                    start=(ci == 0), stop=(ci == nmm - 1))
                                ci += 1
                        nc.tensor.matmul(
                            psA[:], lhsT=xp_sb[:, t, :], rhs=ident_sb[:],
                            start=False, stop=True, skip_group_check=True)
                        ci += 1
                        aggT = workp.tile([128, 128], mybir.dt.bfloat16, tag="aggT")
                        nc.scalar.copy(aggT[:], psA[:])
                        pso = poutp.tile([128, 128], mybir.dt.float32, tag="pout")
                        nc.tensor.matmul(pso[:], lhsT=aggT[:], rhs=w_sb[:, l, :],
                                         start=True, stop=False)
                        # rank-1 accumulate of the bias row: pso += 1 (x) b[l]
                        nc.tensor.matmul(pso[:], lhsT=ones1[:], rhs=b1_sb[:, l, :],
                                         start=False, stop=True, skip_group_check=True)
                        # epilogue: xn = relu(dis*(pso) + prev); x' = fp8(dis*xn)
                        t2 = workp.tile([128, 128], mybir.dt.float32, tag="t2")
                        nc.scalar.activation(
                            out=t2[:], in_=pso[:],
                            func=mybir.ActivationFunctionType.Copy,
                            scale=disT_sb[:, t:t + 1])
                        t3 = workp.tile([128, 128], mybir.dt.float32, tag="t3")
                        nc.vector.tensor_tensor(
                            out=t3[:], in0=t2[:], in1=x_own[:, t, :],
                            op=mybir.AluOpType.add)
                        nc.scalar.activation(
                            out=x_own[:, t, :], in_=t3[:],
                            func=mybir.ActivationFunctionType.Relu)
                        if l < L - 1:
                            nc.scalar.activation(
                                out=xp_sb[:, t, :], in_=x_own[:, t, :],
                                func=mybir.ActivationFunctionType.Copy,
                                scale=disT_sb[:, t:t + 1])
                            nc.sync.dma_start(
                                out=ag_ins[l].rearrange("(t p) f -> p t f", p=128)[:, t, :],
                                in_=xp_sb[:, t, :])
                # the last chunk's AllGather fires at end of layer
                if l < L - 1:
                    k = len(CHUNK_T0) - 1
                    nc.gpsimd.collective_compute(
                        "AllGather",
                        mybir.AluOpType.bypass,
                        replica_groups=[list(range(NCORES))],
                        ins=[ag_ins[l][CHUNK_T0[k] * 128:CHUNK_T1[k] * 128, :].opt()],
                        outs=[tables[l + 1][CHUNK_BASE[k]:CHUNK_BASE[k] + NCORES * RPC[k], :].opt()],
                    )
            nc.sync.dma_start(out=d_out.ap().rearrange("(t p) f -> p t f", p=128),
                              in_=x_own[:])

    nc.compile()
    return nc


def kernel(x, edge_index, W, b):
    global _compiled
    from concourse import bass_utils

    x = np.asarray(x, dtype=np.float32)
    W_np = np.asarray(W, dtype=np.float32)
    b_np = np.asarray(b, dtype=np.float32)
    ei = np.asarray(edge_index)

    meta, per_core = _preprocess(x, ei, W_np, b_np)
    globals()["_last_per_core"] = per_core
    if _compiled is None:
        _compiled = _build(meta)
    nc = _compiled
    res = bass_utils.run_bass_kernel_spmd(nc, per_core, core_ids=list(range(NCORES)))
    out = np.concatenate([res.results[c]["x_out"] for c in range(NCORES)], axis=0)
    return out[:N].astype(np.float32)
